# revision 16
# baseline (speedup 1.0000x reference)
"""Trainium2 Bass kernel for nn_GNN_GRU_83519934038653 (GatedGraphConv-style GNN).

Strategy (8 NeuronCores, SPMD, one NEFF):
  Host: relabel nodes (degree-balanced bins of 128 dst nodes -> smaller T),
  sort edges by dst window, shard by dst node-range (1280 nodes/core), build
  int16 gather indices, per-tile one-hot scatter matrices S, and per-edge
  5-dim features fT = [x_dst-x_src, x_src, 1] (host gather of the tiny x).

  Phase A (device): own-shard h0T = W_emb^T x^T (3 matmuls) -> AllGather
  ([npc,32] payload) -> expand into 256B-row h_full for the gathers.

  Step 0 (fused): per chunk of CT edge-tiles, the edge MLP runs with NO
  device gathers (he = relu(Wf^T fT), ga = relu(W1^T he + b1), wt =
  gaT^T @ W2pa evacuated to SBUF fp16), wt feeds step-0's messages directly
  AND is written to DRAM for steps 1-2.  Message path: dma_gather h[src]
  (split over 4 SWDGE queues -> concurrent Q7 core pairs), DVE multiply
  (h bcast over o) + one pre-reduction add level, PE matmul with one-hot S
  stationary does the dst segment-sum into PSUM windows while the 16
  partial i-lanes ride the moving free dim, per-window tree + transpose,
  GRU, AllGather.

  Steps 1-2: same but wt streams back from DRAM.
"""

import os
import sys

for _p in ("/opt/trn_rl_repo", "/root/.axon_site/_ro/trn_rl_repo"):
    if os.path.isdir(_p) and _p not in sys.path:
        sys.path.insert(0, _p)

import numpy as np

import concourse.bass as bass
import concourse.bacc as bacc
import concourse.mybir as mybir
import concourse.tile as tile
import concourse.bass_utils as bass_utils
from concourse.masks import make_identity

F32 = mybir.dt.float32
F16 = mybir.dt.float16
F8 = mybir.dt.float8e4
I16 = mybir.dt.int16
AF = mybir.ActivationFunctionType
OP = mybir.AluOpType

# problem constants (hardcoded per contest rules)
N_NODES = 10000
N_EDGES = 160000
D = 32
IN_DIM = 2
STEPS = 3
CORES = 8
NPC = 1280  # padded nodes per core; 8*1280 = 10240 >= 10000
WIN = 128   # scatter window (nodes per PSUM accumulation window)

S_DT = F8
S_NP = mybir.dt.np(F8)

LAST_RESULT = None
LAST_META = None


# --------------------------------------------------------------------------- #
# host-side preparation
# --------------------------------------------------------------------------- #

def _wrap_idx(idx, epc):
    """dma_gather index layout: idx j lives at [j % 16, j // 16], replicated to
    128 partitions (each Q7 core pair reads its own 32-partition copy)."""
    w = idx.reshape(epc // 16, 16).T.astype(np.int16)
    return np.tile(w, (8, 1)).copy()


def _balance_nodes(dst, n_nodes, n_pad, nwin):
    """Greedy degree-balanced assignment of nodes to windows of 128 slots.
    Returns perm[orig_node] = new_node so max edges per window is minimized."""
    deg = np.bincount(dst, minlength=n_nodes).astype(np.int64)
    order = np.argsort(-deg, kind="stable")
    import heapq
    cap = n_pad // nwin  # 128 node slots per window
    heap = [(0, w) for w in range(nwin)]
    heapq.heapify(heap)
    count = np.zeros(nwin, np.int64)
    load = np.zeros(nwin, np.int64)
    perm = np.zeros(n_nodes, np.int64)
    for v in order:
        _, w = heapq.heappop(heap)
        perm[v] = w * cap + count[w]
        count[w] += 1
        load[w] += deg[v]
        if count[w] < cap:
            heapq.heappush(heap, (load[w], w))
    return perm, int(load.max())


def host_prep(x, src, dst, W_emb, b_emb, W_theta, b_theta, W_phi, b_phi,
              W1, b1, W2, b2, W_ih, b_ih, W_hh, b_hh,
              n_nodes=N_NODES, npc=NPC, ncores=CORES):
    f32 = np.float32
    f16 = np.float16
    n_pad = npc * ncores
    wpc = npc // WIN
    nwin = ncores * wpc

    src = np.asarray(src).astype(np.int64)
    dst = np.asarray(dst).astype(np.int64)
    x = np.asarray(x, f32)

    perm, _maxload = _balance_nodes(dst, n_nodes, n_pad, nwin)
    src_p = perm[src]
    dst_p = perm[dst]

    order = np.argsort(dst_p, kind="stable")
    src_s = src_p[order]
    dst_s = dst_p[order]
    xs = x[src[order]]          # [E, 2] original-space x gathered on host
    xd = x[dst[order]]

    win_of_edge = dst_s // WIN
    counts = np.bincount(win_of_edge, minlength=nwin)
    T = max(1, int(np.ceil(counts.max() / 128)))
    ntpc = wpc * T
    epc = ntpc * 128

    win_start = np.concatenate([[0], np.cumsum(counts)])

    per_core = []
    for c in range(ncores):
        src_idx = np.zeros(epc, np.int64)
        S = np.zeros((128, ntpc, 128), np.float32)
        fT = np.zeros((5, epc), f32)
        for w in range(wpc):
            g = c * wpc + w
            e0, e1 = int(win_start[g]), int(win_start[g + 1])
            k = e1 - e0
            if k == 0:
                continue
            base = w * T * 128
            j = np.arange(k)
            src_idx[base + j] = src_s[e0:e1]
            dloc = dst_s[e0:e1] - (c * npc + w * WIN)
            assert (dloc >= 0).all() and (dloc < WIN).all()
            S[j % 128, w * T + j // 128, dloc] = 1.0
            fT[0:2, base + j] = (xd[e0:e1] - xs[e0:e1]).T
            fT[2:4, base + j] = xs[e0:e1].T
            fT[4, base + j] = 1.0
        per_core.append({
            "src_w": _wrap_idx(src_idx, epc),
            "s_mat": S.astype(S_NP),
            "fT": fT.astype(f16),
            "xTo_own": None,
        })

    x_perm = np.zeros((n_pad, IN_DIM), f32)
    x_perm[perm] = x[:n_nodes]
    xT_aug = np.concatenate([x_perm.T, np.ones((1, n_pad), f32)], 0)
    for c in range(ncores):
        per_core[c]["xTo_own"] = np.ascontiguousarray(
            xT_aug[:, c * npc:(c + 1) * npc])

    W_emb = np.asarray(W_emb, f32); b_emb = np.asarray(b_emb, f32)
    W_theta = np.asarray(W_theta, f32); W_phi = np.asarray(W_phi, f32)

    W_emb_aug = np.concatenate([W_emb, b_emb[None, :]], 0)  # [3, 32]

    # EdgeConv folded: he = relu([xd-xs, xs, 1] @ Wf)
    cvec = b_emb @ W_phi + np.asarray(b_theta, f32) + np.asarray(b_phi, f32)
    Wf = np.concatenate([W_emb @ W_theta, W_emb @ W_phi, cvec[None, :]], 0)  # [5,32]

    W2p = np.asarray(W2, f32).reshape(64, D, D).transpose(0, 2, 1).reshape(64, D * D)
    b2p = np.asarray(b2, f32).reshape(D, D).T.reshape(D * D)
    W2pa = np.concatenate([W2p, b2p[None, :]], 0).astype(f16)  # [65, 1024]

    shared = {
        "wemb": W_emb_aug,
        "wf": Wf.astype(f16),
        "w1": np.asarray(W1, f32).astype(f16),
        "w2pa": W2pa,
        "w_ih": np.asarray(W_ih, f32),
        "w_hh": np.asarray(W_hh, f32),
        "b1c": np.asarray(b1, f32)[:, None],
        "b_r": (np.asarray(b_ih, f32)[0:D] + np.asarray(b_hh, f32)[0:D])[:, None],
        "b_z": (np.asarray(b_ih, f32)[D:2 * D] + np.asarray(b_hh, f32)[D:2 * D])[:, None],
        "b_in": np.asarray(b_ih, f32)[2 * D:3 * D][:, None],
        "b_hn": np.asarray(b_hh, f32)[2 * D:3 * D][:, None],
    }
    meta = dict(T=T, npc=npc, ncores=ncores, n_pad=n_pad, wpc=wpc,
                ntpc=ntpc, epc=epc, steps=STEPS)
    return shared, per_core, meta, perm


# --------------------------------------------------------------------------- #
# device kernel builder
# --------------------------------------------------------------------------- #

def _bcast_mid(ap_base, count):
    """Insert a [0, count] broadcast dim before the innermost dim of an AP."""
    aps = [list(p) for p in ap_base.ap]
    new = aps[:-1] + [[0, count]] + [aps[-1]]
    return bass.AP(ap_base.tensor, ap_base.offset, new)


def build_nc(meta):
    T = meta["T"]; npc = meta["npc"]; ncores = meta["ncores"]
    n_pad = meta["n_pad"]; wpc = meta["wpc"]; ntpc = meta["ntpc"]
    epc = meta["epc"]; steps = meta["steps"]
    steps_exec = int(os.environ.get("K_STEPS", steps))
    CT = int(os.environ.get("K_CT", "6"))        # tiles per prod chunk
    NQ = int(os.environ.get("K_NQ", "4"))        # SWDGE queues for gathers
    NSPLIT = int(os.environ.get("K_NSPLIT", "8"))  # gather instructions/step
    P16_POOL = int(os.environ.get("K_P16_POOL", "1"))  # p16 adds on GpSimd

    nc = bacc.Bacc("TRN2", target_bir_lowering=False, debug=False,
                   enable_asserts=False, num_devices=ncores,
                   num_swdge_queues=NQ)

    # ---- I/O tensors ----
    t_xTo = nc.dram_tensor("xTo_own", [IN_DIM + 1, npc], F32, kind="ExternalInput")
    t_wemb = nc.dram_tensor("wemb", [IN_DIM + 1, D], F32, kind="ExternalInput")
    t_wf = nc.dram_tensor("wf", [5, D], F16, kind="ExternalInput")
    t_w1 = nc.dram_tensor("w1", [D, 64], F16, kind="ExternalInput")
    t_w2 = nc.dram_tensor("w2pa", [65, 1024], F16, kind="ExternalInput")
    t_wih = nc.dram_tensor("w_ih", [D, 3 * D], F32, kind="ExternalInput")
    t_whh = nc.dram_tensor("w_hh", [D, 3 * D], F32, kind="ExternalInput")
    t_b1c = nc.dram_tensor("b1c", [64, 1], F32, kind="ExternalInput")
    t_br = nc.dram_tensor("b_r", [D, 1], F32, kind="ExternalInput")
    t_bz = nc.dram_tensor("b_z", [D, 1], F32, kind="ExternalInput")
    t_bin = nc.dram_tensor("b_in", [D, 1], F32, kind="ExternalInput")
    t_bhn = nc.dram_tensor("b_hn", [D, 1], F32, kind="ExternalInput")
    t_fT = nc.dram_tensor("fT", [5, epc], F16, kind="ExternalInput")
    t_srcw = nc.dram_tensor("src_w", [128, epc // 16], I16, kind="ExternalInput")
    t_smat = nc.dram_tensor("s_mat", [128, ntpc, 128], S_DT, kind="ExternalInput")
    t_out = nc.dram_tensor("out_h", [npc, D], F32, kind="ExternalOutput")

    with tile.TileContext(nc) as tc:
        with tc.tile_pool(name="dram", bufs=1, space="DRAM") as dpool, \
             tc.tile_pool(name="const", bufs=1) as cpool:
            we_dram = dpool.tile([epc, 1024], F16, name="we_dram")
            h_fulls = [dpool.tile([n_pad, 128], F16, addr_space="Shared",
                                  name=f"h_full{s}") for s in range(steps)]
            cc_ins = [dpool.tile([npc, 128], F16, name=f"cc_in{s}")
                      for s in range(steps)]

            # resident constants
            idm = cpool.tile([128, 128], F32, name="idm")
            make_identity(nc, idm[:])
            S_sb = cpool.tile([128, ntpc * 128], S_DT, name="S_sb")
            nc.sync.dma_start(S_sb[:], t_smat.ap().rearrange("p t e -> p (t e)"))
            isrc = cpool.tile([128, epc // 16], I16, name="isrc")
            nc.sync.dma_start(isrc[:], t_srcw.ap())

            def load_const(t, shape, dtype, name):
                s = cpool.tile(shape, dtype, name=name)
                nc.sync.dma_start(s[:], t.ap())
                return s

            xTo_sb = load_const(t_xTo, [IN_DIM + 1, npc], F32, "xTo_sb")
            wemb_sb = load_const(t_wemb, [IN_DIM + 1, D], F32, "wemb_sb")
            wf_sb = load_const(t_wf, [5, D], F16, "wf_sb")
            w1_sb = load_const(t_w1, [D, 64], F16, "w1_sb")
            w2_sb = load_const(t_w2, [65, 1024], F16, "w2_sb")
            wih_sb = load_const(t_wih, [D, 3 * D], F32, "wih_sb")
            whh_sb = load_const(t_whh, [D, 3 * D], F32, "whh_sb")
            b1c_sb = load_const(t_b1c, [64, 1], F32, "b1c_sb")
            br_sb = load_const(t_br, [D, 1], F32, "br_sb")
            bz_sb = load_const(t_bz, [D, 1], F32, "bz_sb")
            bin_sb = load_const(t_bin, [D, 1], F32, "bin_sb")
            bhn_sb = load_const(t_bhn, [D, 1], F32, "bhn_sb")

            h_bufs = [cpool.tile([D, npc], F32, name=f"hT{i}") for i in range(2)]

            we_view = we_dram[:].rearrange("(t p) f -> p t f", p=128)

            # shared pools for all phases (PSUM: 2+2+2+2 = 8 banks)
            with tc.tile_pool(name="pMM", bufs=2, space="PSUM") as pmm, \
                 tc.tile_pool(name="pW", bufs=2, space="PSUM") as pw, \
                 tc.tile_pool(name="pSA", bufs=2, space="PSUM") as psa_pool, \
                 tc.tile_pool(name="pT", bufs=1, space="PSUM") as ppt, \
                 tc.tile_pool(name="sG", bufs=1) as sg, \
                 tc.tile_pool(name="sWq", bufs=2) as swq, \
                 tc.tile_pool(name="sPr", bufs=2) as spr, \
                 tc.tile_pool(name="sP16", bufs=2) as sp16, \
                 tc.tile_pool(name="sWin", bufs=2) as swin, \
                 tc.tile_pool(name="sGru", bufs=1) as sgru, \
                 tc.tile_pool(name="sA2", bufs=2) as sa2, \
                 tc.tile_pool(name="sFt", bufs=2) as sft, \
                 tc.tile_pool(name="sSt", bufs=1) as sst:

                def distribute_h(hT, step):
                    """transpose hT [D, npc] -> [npc, D] rows, DMA to cc_in,
                    AllGather into h_fulls[step]."""
                    hst = sst.tile([128, wpc, D], F16, tag="hst")
                    for w in range(wpc):
                        ps2 = ppt.tile([128, 128], F32, tag="ps2")
                        nc.tensor.transpose(ps2[:, 0:D],
                                            hT[:, w * 128:(w + 1) * 128],
                                            idm[0:D, 0:D])
                        nc.scalar.copy(hst[:, w, :], ps2[:, 0:D])
                    nc.sync.dma_start(
                        cc_ins[step][:, 0:D].rearrange("(w p) d -> p w d", p=128),
                        hst[:])
                    nc.gpsimd.collective_compute(
                        "AllGather", OP.bypass,
                        replica_groups=[list(range(ncores))],
                        ins=[cc_ins[step][:].opt()],
                        outs=[h_fulls[step][:].opt()])

                def issue_gathers(G, step):
                    bnds = [round(i * ntpc / NSPLIT) for i in range(NSPLIT + 1)]
                    for i, (ta, tb) in enumerate(zip(bnds[:-1], bnds[1:])):
                        if tb > ta:
                            nc.gpsimd.dma_gather(
                                G[:, ta:tb, :], h_fulls[step][:, :],
                                isrc[:, ta * 8:tb * 8],
                                (tb - ta) * 128, (tb - ta) * 128, 128,
                                transpose=False, single_packet=False,
                                queue_num=i % NQ)

                def scatter_tile(gt, p16, j, aT, state):
                    """accumulate tile gt into its PSUM window; on window end
                    run the i-reduction tree + transpose into aT.  Returns the
                    completed window index (or None)."""
                    w = gt // T
                    tloc = gt % T
                    if tloc == 0:
                        psa_new = psa_pool.tile([128, 512], F32, tag="psa")
                        state["psa"] = psa_new
                    psa = state["psa"]
                    nc.tensor.matmul(
                        psa[:], lhsT=S_sb[:, gt * 128:(gt + 1) * 128],
                        rhs=p16[:, j, :, :],
                        start=(tloc == 0), stop=(tloc == T - 1))
                    if tloc != T - 1:
                        return None
                    aw = swin.tile([128, D, 16], F32, tag="aw")
                    nc.scalar.copy(
                        aw[:], psa[:].rearrange("p (o i) -> p o i", o=D))
                    t8 = swin.tile([128, D, 8], F32, tag="t8")
                    nc.vector.tensor_tensor(t8[:], aw[:, :, 0:8],
                                            aw[:, :, 8:16], op=OP.add)
                    t4 = swin.tile([128, D, 4], F32, tag="t4")
                    nc.vector.tensor_tensor(t4[:], t8[:, :, 0:4],
                                            t8[:, :, 4:8], op=OP.add)
                    t2 = swin.tile([128, D, 2], F32, tag="t2")
                    nc.vector.tensor_tensor(t2[:], t4[:, :, 0:2],
                                            t4[:, :, 2:4], op=OP.add)
                    t1 = swin.tile([128, D], F32, tag="t1")
                    nc.vector.tensor_tensor(t1[:], t2[:, :, 0],
                                            t2[:, :, 1], op=OP.add)
                    pst = ppt.tile([D, 128], F32, tag="pst")
                    nc.tensor.transpose(pst[:], t1[:], idm[:])
                    nc.vector.tensor_copy(
                        aT[:, w * 128:(w + 1) * 128], pst[:])
                    return w

                def gru_chunk(aT, h_cur, h_new, c0, cn):
                    if True:
                        cs = slice(c0, c0 + cn)
                        pgi = pmm.tile([3 * D, 512], F32, tag="mm96")
                        nc.tensor.matmul(pgi[:, 0:cn], lhsT=wih_sb[:],
                                         rhs=aT[:, cs], start=True, stop=True)
                        pgh = pmm.tile([3 * D, 512], F32, tag="mm96")
                        nc.tensor.matmul(pgh[:, 0:cn], lhsT=whh_sb[:],
                                         rhs=h_cur[:, cs], start=True, stop=True)
                        gh_sb = sgru.tile([3 * D, 512], F32, tag="gh_sb")
                        nc.scalar.copy(gh_sb[:, 0:cn], pgh[:, 0:cn])
                        tr = sgru.tile([D, 512], F32, tag="tr")
                        nc.vector.tensor_add(tr[:, 0:cn], pgi[0:D, 0:cn],
                                             gh_sb[0:D, 0:cn])
                        r = sgru.tile([D, 512], F32, tag="r")
                        nc.scalar.activation(r[:, 0:cn], tr[:, 0:cn], AF.Sigmoid,
                                             bias=br_sb[:])
                        tz = sgru.tile([D, 512], F32, tag="tz")
                        nc.vector.tensor_add(tz[:, 0:cn], pgi[D:2 * D, 0:cn],
                                             gh_sb[D:2 * D, 0:cn])
                        z = sgru.tile([D, 512], F32, tag="z")
                        nc.scalar.activation(z[:, 0:cn], tz[:, 0:cn], AF.Sigmoid,
                                             bias=bz_sb[:])
                        hnb = sgru.tile([D, 512], F32, tag="hnb")
                        nc.vector.tensor_scalar_add(hnb[:, 0:cn],
                                                    gh_sb[2 * D:3 * D, 0:cn],
                                                    bhn_sb[:])
                        rhn = sgru.tile([D, 512], F32, tag="rhn")
                        nc.vector.tensor_mul(rhn[:, 0:cn], r[:, 0:cn], hnb[:, 0:cn])
                        tn_ = sgru.tile([D, 512], F32, tag="tn_")
                        nc.vector.tensor_add(tn_[:, 0:cn], rhn[:, 0:cn],
                                             pgi[2 * D:3 * D, 0:cn])
                        ngate = sgru.tile([D, 512], F32, tag="ngate")
                        nc.scalar.activation(ngate[:, 0:cn], tn_[:, 0:cn], AF.Tanh,
                                             bias=bin_sb[:])
                        hmn = sgru.tile([D, 512], F32, tag="hmn")
                        nc.vector.tensor_sub(hmn[:, 0:cn], h_cur[:, cs],
                                             ngate[:, 0:cn])
                        zh = sgru.tile([D, 512], F32, tag="zh")
                        nc.vector.tensor_mul(zh[:, 0:cn], z[:, 0:cn], hmn[:, 0:cn])
                        nc.vector.tensor_add(h_new[:, cs], ngate[:, 0:cn],
                                             zh[:, 0:cn])

                def on_window_done(w, aT, h_cur, h_new):
                    """run the GRU for a 512-col slab once its 4 windows of
                    aT are final (keeps the GRU off the step tail)."""
                    if w is None:
                        return
                    if (w + 1) % 4 == 0:
                        gru_chunk(aT, h_cur, h_new, (w - 3) * 128, 512)
                    elif w == wpc - 1 and wpc % 4:
                        gru_chunk(aT, h_cur, h_new, (wpc - wpc % 4) * 128,
                                  (wpc % 4) * 128)

                def p16_add(p16, prod, k):
                    eng = nc.gpsimd if P16_POOL else nc.vector
                    eng.tensor_tensor(p16[:, 0:k, :, :], prod[:, 0:k, :, 0:16],
                                      prod[:, 0:k, :, 16:32], op=OP.add)

                # ---------------- Phase A1: own-shard h0T + AllGather -------
                for c0 in range(0, npc, 512):
                    cn = min(512, npc - c0)
                    ps = pmm.tile([3 * D, 512], F32, tag="mm96")
                    nc.tensor.matmul(ps[0:D, 0:cn], lhsT=wemb_sb[:],
                                     rhs=xTo_sb[:, c0:c0 + cn],
                                     start=True, stop=True)
                    nc.vector.tensor_copy(h_bufs[0][:, c0:c0 + cn],
                                          ps[0:D, 0:cn])
                distribute_h(h_bufs[0], 0)

                # ---------------- Step 0 (fused edge-MLP + messages) --------
                G = sg.tile([128, ntpc, 128], F16, tag="G")
                issue_gathers(G, 0)

                aT = sgru.tile([D, npc], F32, tag="aT")
                state = {}
                evac_flip = 0
                for q0 in range(0, ntpc, CT):
                    k = min(CT, ntpc - q0)
                    wt = swq.tile([128, CT, 1024], F16, tag="wq")
                    # edge MLP for this chunk (k*128 edges, groups of 512)
                    for g0 in range(0, k * 128, 512):
                        gn = min(512, k * 128 - g0)
                        e0 = q0 * 128 + g0
                        fts = sft.tile([5, 512], F16, tag="fts")
                        nc.sync.dma_start(fts[:, 0:gn], t_fT.ap()[:, e0:e0 + gn])
                        psh = pmm.tile([3 * D, 512], F32, tag="mm96")
                        nc.tensor.matmul(psh[0:D, 0:gn], lhsT=wf_sb[:],
                                         rhs=fts[:, 0:gn], start=True, stop=True)
                        he = sa2.tile([D, 512], F16, tag="he")
                        nc.scalar.activation(he[:, 0:gn], psh[0:D, 0:gn], AF.Relu)
                        psg = pmm.tile([3 * D, 512], F32, tag="mm96")
                        nc.tensor.matmul(psg[0:64, 0:gn], lhsT=w1_sb[:],
                                         rhs=he[:, 0:gn], start=True, stop=True)
                        ga = sa2.tile([65, 512], F16, tag="ga")
                        nc.vector.memset(ga[64:65, 0:gn], 1.0)
                        nc.scalar.activation(ga[0:64, 0:gn], psg[0:64, 0:gn],
                                             AF.Relu, bias=b1c_sb[:])
                        for s0 in range(0, gn, 128):
                            jt = (g0 + s0) // 128  # tile within chunk
                            for half in range(2):
                                pwt = pw.tile([128, 512], F32, tag="pw")
                                nc.tensor.matmul(
                                    pwt[:], lhsT=ga[:, s0:s0 + 128],
                                    rhs=w2_sb[:, half * 512:(half + 1) * 512],
                                    start=True, stop=True)
                                dst_ap = wt[:, jt, half * 512:(half + 1) * 512]
                                if evac_flip == 0:
                                    nc.scalar.copy(dst_ap, pwt[:])
                                else:
                                    nc.vector.tensor_copy(dst_ap, pwt[:])
                                evac_flip ^= 1
                    nc.sync.dma_start(we_view[:, q0:q0 + k, :], wt[:, 0:k, :])
                    # message path for this chunk
                    prod = spr.tile([128, CT, D, D], F16, tag="prod")
                    in1 = _bcast_mid(G[:, q0:q0 + k, 0:D], D)
                    nc.vector.tensor_tensor(
                        prod[:, 0:k, :, :],
                        wt[:, 0:k, :].rearrange("p t (o i) -> p t o i", o=D),
                        in1, op=OP.mult)
                    p16 = sp16.tile([128, CT, D, 16], F16, tag="p16")
                    p16_add(p16, prod, k)
                    for j in range(k):
                        wdone = scatter_tile(q0 + j, p16, j, aT, state)
                        on_window_done(wdone, aT, h_bufs[0], h_bufs[1])

                if steps_exec > 1:
                    distribute_h(h_bufs[1], 1)

                # ---------------- Steps 1..: stream We from DRAM ------------
                for step in range(1, steps_exec):
                    h_cur = h_bufs[step % 2]
                    h_new = h_bufs[(step + 1) % 2]

                    G = sg.tile([128, ntpc, 128], F16, tag="G")
                    issue_gathers(G, step)

                    aT = sgru.tile([D, npc], F32, tag="aT")
                    state = {}
                    for q0 in range(0, ntpc, CT):
                        k = min(CT, ntpc - q0)
                        wq = swq.tile([128, CT, 1024], F16, tag="wq")
                        nc.sync.dma_start(wq[:, 0:k, :], we_view[:, q0:q0 + k, :])
                        prod = spr.tile([128, CT, D, D], F16, tag="prod")
                        in1 = _bcast_mid(G[:, q0:q0 + k, 0:D], D)
                        nc.vector.tensor_tensor(
                            prod[:, 0:k, :, :],
                            wq[:, 0:k, :].rearrange("p t (o i) -> p t o i", o=D),
                            in1, op=OP.mult)
                        p16 = sp16.tile([128, CT, D, 16], F16, tag="p16")
                        p16_add(p16, prod, k)
                        for j in range(k):
                            wdone = scatter_tile(q0 + j, p16, j, aT, state)
                            on_window_done(wdone, aT, h_cur, h_new)

                    if step < steps_exec - 1:
                        distribute_h(h_new, step + 1)

                # ---------------- final store -------------------------------
                h_fin = h_bufs[steps_exec % 2]
                ost = sgru.tile([128, wpc, D], F32, tag="ost")
                for w in range(wpc):
                    ps2 = ppt.tile([128, 128], F32, tag="ps2")
                    nc.tensor.transpose(ps2[:, 0:D],
                                        h_fin[:, w * 128:(w + 1) * 128],
                                        idm[0:D, 0:D])
                    nc.scalar.copy(ost[:, w, :], ps2[:, 0:D])
                nc.sync.dma_start(
                    t_out.ap().rearrange("(w p) d -> p w d", p=128), ost[:])

    nc.compile()
    return nc


# --------------------------------------------------------------------------- #
# entry point
# --------------------------------------------------------------------------- #

def run(inputs, n_nodes=N_NODES, npc=NPC, **spmd_kwargs):
    global LAST_RESULT, LAST_META
    shared, per_core, meta, perm = host_prep(**inputs, n_nodes=n_nodes, npc=npc)
    LAST_META = meta
    nc = build_nc(meta)
    in_maps = [dict(shared, **pc) for pc in per_core]
    res = bass_utils.run_bass_kernel_spmd(
        nc, in_maps, core_ids=list(range(meta["ncores"])), **spmd_kwargs)
    LAST_RESULT = res
    out = np.concatenate([res.results[c]["out_h"] for c in range(meta["ncores"])], 0)
    return np.ascontiguousarray(out[perm]).astype(np.float32)


def kernel(**inputs):
    return run(inputs)


# revision 17
# speedup vs baseline: 1.2417x; 1.2417x over previous
"""Trainium2 Bass kernel for nn_GNN_GRU_83519934038653 (GatedGraphConv-style GNN).

Strategy (8 NeuronCores, SPMD, one NEFF):
  Host: relabel nodes (degree-balanced bins of 128 dst nodes -> smaller T),
  sort edges by dst window, shard by dst node-range (1280 nodes/core), build
  int16 gather indices, per-tile one-hot scatter matrices S, and per-edge
  5-dim features fT = [x_dst-x_src, x_src, 1] (host gather of the tiny x).

  Phase A (device): own-shard h0T = W_emb^T x^T (3 matmuls) -> AllGather
  ([npc,32] payload) -> expand into 256B-row h_full for the gathers.

  Step 0 (fused): per chunk of CT edge-tiles, the edge MLP runs with NO
  device gathers (he = relu(Wf^T fT), ga = relu(W1^T he + b1), wt =
  gaT^T @ W2pa evacuated to SBUF fp16), wt feeds step-0's messages directly
  AND is written to DRAM for steps 1-2.  Message path: dma_gather h[src]
  (split over 4 SWDGE queues -> concurrent Q7 core pairs), DVE multiply
  (h bcast over o) + one pre-reduction add level, PE matmul with one-hot S
  stationary does the dst segment-sum into PSUM windows while the 16
  partial i-lanes ride the moving free dim, per-window tree + transpose,
  GRU, AllGather.

  Steps 1-2: same but wt streams back from DRAM.
"""

import os
import sys

for _p in ("/opt/trn_rl_repo", "/root/.axon_site/_ro/trn_rl_repo"):
    if os.path.isdir(_p) and _p not in sys.path:
        sys.path.insert(0, _p)

import numpy as np

import concourse.bass as bass
import concourse.bacc as bacc
import concourse.mybir as mybir
import concourse.tile as tile
import concourse.bass_utils as bass_utils
from concourse.masks import make_identity

F32 = mybir.dt.float32
F16 = mybir.dt.float16
F8 = mybir.dt.float8e4
I16 = mybir.dt.int16
AF = mybir.ActivationFunctionType
OP = mybir.AluOpType

# problem constants (hardcoded per contest rules)
N_NODES = 10000
N_EDGES = 160000
D = 32
IN_DIM = 2
STEPS = 3
CORES = 8
NPC = 1280  # padded nodes per core; 8*1280 = 10240 >= 10000
WIN = 128   # scatter window (nodes per PSUM accumulation window)

S_DT = F8
S_NP = mybir.dt.np(F8)

LAST_RESULT = None
LAST_META = None


# --------------------------------------------------------------------------- #
# host-side preparation
# --------------------------------------------------------------------------- #

def _wrap_idx(idx, epc):
    """dma_gather index layout: idx j lives at [j % 16, j // 16], replicated to
    128 partitions (each Q7 core pair reads its own 32-partition copy)."""
    w = idx.reshape(epc // 16, 16).T.astype(np.int16)
    return np.tile(w, (8, 1)).copy()


def _balance_nodes(dst, n_nodes, n_pad, nwin):
    """Greedy degree-balanced assignment of nodes to windows of 128 slots.
    Returns perm[orig_node] = new_node so max edges per window is minimized."""
    deg = np.bincount(dst, minlength=n_nodes).astype(np.int64)
    order = np.argsort(-deg, kind="stable")
    import heapq
    cap = n_pad // nwin  # 128 node slots per window
    heap = [(0, w) for w in range(nwin)]
    heapq.heapify(heap)
    count = np.zeros(nwin, np.int64)
    load = np.zeros(nwin, np.int64)
    perm = np.zeros(n_nodes, np.int64)
    for v in order:
        _, w = heapq.heappop(heap)
        perm[v] = w * cap + count[w]
        count[w] += 1
        load[w] += deg[v]
        if count[w] < cap:
            heapq.heappush(heap, (load[w], w))
    return perm, int(load.max())


def host_prep(x, src, dst, W_emb, b_emb, W_theta, b_theta, W_phi, b_phi,
              W1, b1, W2, b2, W_ih, b_ih, W_hh, b_hh,
              n_nodes=N_NODES, npc=NPC, ncores=CORES):
    f32 = np.float32
    f16 = np.float16
    n_pad = npc * ncores
    wpc = npc // WIN
    nwin = ncores * wpc

    src = np.asarray(src).astype(np.int64)
    dst = np.asarray(dst).astype(np.int64)
    x = np.asarray(x, f32)

    perm, _maxload = _balance_nodes(dst, n_nodes, n_pad, nwin)
    src_p = perm[src]
    dst_p = perm[dst]

    order = np.argsort(dst_p, kind="stable")
    src_s = src_p[order]
    dst_s = dst_p[order]
    xs = x[src[order]]          # [E, 2] original-space x gathered on host
    xd = x[dst[order]]

    win_of_edge = dst_s // WIN
    counts = np.bincount(win_of_edge, minlength=nwin)
    T = max(1, int(np.ceil(counts.max() / 128)))
    ntpc = wpc * T
    epc = ntpc * 128

    win_start = np.concatenate([[0], np.cumsum(counts)])

    per_core = []
    for c in range(ncores):
        src_idx = np.zeros(epc, np.int64)
        S = np.zeros((128, ntpc, 128), np.float32)
        fT = np.zeros((5, epc), f32)
        for w in range(wpc):
            g = c * wpc + w
            e0, e1 = int(win_start[g]), int(win_start[g + 1])
            k = e1 - e0
            if k == 0:
                continue
            base = w * T * 128
            j = np.arange(k)
            src_idx[base + j] = src_s[e0:e1]
            dloc = dst_s[e0:e1] - (c * npc + w * WIN)
            assert (dloc >= 0).all() and (dloc < WIN).all()
            S[j % 128, w * T + j // 128, dloc] = 1.0
            fT[0:2, base + j] = (xd[e0:e1] - xs[e0:e1]).T
            fT[2:4, base + j] = xs[e0:e1].T
            fT[4, base + j] = 1.0
        per_core.append({
            "src_w": _wrap_idx(src_idx, epc),
            "s_mat": S.astype(S_NP),
            "fT": fT.astype(f16),
            "xTo_own": None,
        })

    x_perm = np.zeros((n_pad, IN_DIM), f32)
    x_perm[perm] = x[:n_nodes]
    xT_aug = np.concatenate([x_perm.T, np.ones((1, n_pad), f32)], 0)
    for c in range(ncores):
        per_core[c]["xTo_own"] = np.ascontiguousarray(
            xT_aug[:, c * npc:(c + 1) * npc])

    W_emb = np.asarray(W_emb, f32); b_emb = np.asarray(b_emb, f32)
    W_theta = np.asarray(W_theta, f32); W_phi = np.asarray(W_phi, f32)

    W_emb_aug = np.concatenate([W_emb, b_emb[None, :]], 0)  # [3, 32]

    # EdgeConv folded: he = relu([xd-xs, xs, 1] @ Wf)
    cvec = b_emb @ W_phi + np.asarray(b_theta, f32) + np.asarray(b_phi, f32)
    Wf = np.concatenate([W_emb @ W_theta, W_emb @ W_phi, cvec[None, :]], 0)  # [5,32]

    W2p = np.asarray(W2, f32).reshape(64, D, D).transpose(0, 2, 1).reshape(64, D * D)
    b2p = np.asarray(b2, f32).reshape(D, D).T.reshape(D * D)
    W2pa = np.concatenate([W2p, b2p[None, :]], 0).astype(f16)  # [65, 1024]

    shared = {
        "wemb": W_emb_aug,
        "wf": Wf.astype(f16),
        "w1": np.asarray(W1, f32).astype(f16),
        "w2pa": W2pa,
        "w_ih": np.asarray(W_ih, f32),
        "w_hh": np.asarray(W_hh, f32),
        "b1c": np.asarray(b1, f32)[:, None],
        "b_r": (np.asarray(b_ih, f32)[0:D] + np.asarray(b_hh, f32)[0:D])[:, None],
        "b_z": (np.asarray(b_ih, f32)[D:2 * D] + np.asarray(b_hh, f32)[D:2 * D])[:, None],
        "b_in": np.asarray(b_ih, f32)[2 * D:3 * D][:, None],
        "b_hn": np.asarray(b_hh, f32)[2 * D:3 * D][:, None],
    }
    meta = dict(T=T, npc=npc, ncores=ncores, n_pad=n_pad, wpc=wpc,
                ntpc=ntpc, epc=epc, steps=STEPS)
    return shared, per_core, meta, perm


# --------------------------------------------------------------------------- #
# device kernel builder
# --------------------------------------------------------------------------- #

def _bcast_mid(ap_base, count):
    """Insert a [0, count] broadcast dim before the innermost dim of an AP."""
    aps = [list(p) for p in ap_base.ap]
    new = aps[:-1] + [[0, count]] + [aps[-1]]
    return bass.AP(ap_base.tensor, ap_base.offset, new)


def build_nc(meta):
    T = meta["T"]; npc = meta["npc"]; ncores = meta["ncores"]
    n_pad = meta["n_pad"]; wpc = meta["wpc"]; ntpc = meta["ntpc"]
    epc = meta["epc"]; steps = meta["steps"]
    steps_exec = int(os.environ.get("K_STEPS", steps))
    CT = int(os.environ.get("K_CT", "6"))        # tiles per prod chunk
    NQ = int(os.environ.get("K_NQ", "4"))        # SWDGE queues for gathers
    NSPLIT = int(os.environ.get("K_NSPLIT", "8"))  # gather instructions/step
    P16_POOL = int(os.environ.get("K_P16_POOL", "0"))  # p16 adds on GpSimd

    nc = bacc.Bacc("TRN2", target_bir_lowering=False, debug=False,
                   enable_asserts=False, num_devices=ncores,
                   num_swdge_queues=NQ)

    # ---- I/O tensors ----
    t_xTo = nc.dram_tensor("xTo_own", [IN_DIM + 1, npc], F32, kind="ExternalInput")
    t_wemb = nc.dram_tensor("wemb", [IN_DIM + 1, D], F32, kind="ExternalInput")
    t_wf = nc.dram_tensor("wf", [5, D], F16, kind="ExternalInput")
    t_w1 = nc.dram_tensor("w1", [D, 64], F16, kind="ExternalInput")
    t_w2 = nc.dram_tensor("w2pa", [65, 1024], F16, kind="ExternalInput")
    t_wih = nc.dram_tensor("w_ih", [D, 3 * D], F32, kind="ExternalInput")
    t_whh = nc.dram_tensor("w_hh", [D, 3 * D], F32, kind="ExternalInput")
    t_b1c = nc.dram_tensor("b1c", [64, 1], F32, kind="ExternalInput")
    t_br = nc.dram_tensor("b_r", [D, 1], F32, kind="ExternalInput")
    t_bz = nc.dram_tensor("b_z", [D, 1], F32, kind="ExternalInput")
    t_bin = nc.dram_tensor("b_in", [D, 1], F32, kind="ExternalInput")
    t_bhn = nc.dram_tensor("b_hn", [D, 1], F32, kind="ExternalInput")
    t_fT = nc.dram_tensor("fT", [5, epc], F16, kind="ExternalInput")
    t_srcw = nc.dram_tensor("src_w", [128, epc // 16], I16, kind="ExternalInput")
    t_smat = nc.dram_tensor("s_mat", [128, ntpc, 128], S_DT, kind="ExternalInput")
    t_out = nc.dram_tensor("out_h", [npc, D], F32, kind="ExternalOutput")

    with tile.TileContext(nc) as tc:
        with tc.tile_pool(name="dram", bufs=1, space="DRAM") as dpool, \
             tc.tile_pool(name="const", bufs=1) as cpool:
            we_dram = dpool.tile([epc, 1024], F16, name="we_dram")
            h_fulls = [dpool.tile([n_pad, 128], F16, addr_space="Shared",
                                  name=f"h_full{s}") for s in range(steps)]
            cc_ins = [dpool.tile([npc, 128], F16, name=f"cc_in{s}")
                      for s in range(steps)]

            # resident constants
            idm = cpool.tile([128, 128], F32, name="idm")
            make_identity(nc, idm[:])
            S_sb = cpool.tile([128, ntpc * 128], S_DT, name="S_sb")
            nc.sync.dma_start(S_sb[:], t_smat.ap().rearrange("p t e -> p (t e)"))
            isrc = cpool.tile([128, epc // 16], I16, name="isrc")
            nc.sync.dma_start(isrc[:], t_srcw.ap())

            def load_const(t, shape, dtype, name):
                s = cpool.tile(shape, dtype, name=name)
                nc.sync.dma_start(s[:], t.ap())
                return s

            xTo_sb = load_const(t_xTo, [IN_DIM + 1, npc], F32, "xTo_sb")
            wemb_sb = load_const(t_wemb, [IN_DIM + 1, D], F32, "wemb_sb")
            wf_sb = load_const(t_wf, [5, D], F16, "wf_sb")
            w1_sb = load_const(t_w1, [D, 64], F16, "w1_sb")
            w2_sb = load_const(t_w2, [65, 1024], F16, "w2_sb")
            wih_sb = load_const(t_wih, [D, 3 * D], F32, "wih_sb")
            whh_sb = load_const(t_whh, [D, 3 * D], F32, "whh_sb")
            b1c_sb = load_const(t_b1c, [64, 1], F32, "b1c_sb")
            br_sb = load_const(t_br, [D, 1], F32, "br_sb")
            bz_sb = load_const(t_bz, [D, 1], F32, "bz_sb")
            bin_sb = load_const(t_bin, [D, 1], F32, "bin_sb")
            bhn_sb = load_const(t_bhn, [D, 1], F32, "bhn_sb")

            h_bufs = [cpool.tile([D, npc], F32, name=f"hT{i}") for i in range(2)]

            we_view = we_dram[:].rearrange("(t p) f -> p t f", p=128)

            # shared pools for all phases (PSUM: 2+2+2+2 = 8 banks)
            with tc.tile_pool(name="pMM", bufs=2, space="PSUM") as pmm, \
                 tc.tile_pool(name="pW", bufs=2, space="PSUM") as pw, \
                 tc.tile_pool(name="pSA", bufs=2, space="PSUM") as psa_pool, \
                 tc.tile_pool(name="pT", bufs=1, space="PSUM") as ppt, \
                 tc.tile_pool(name="sG", bufs=1) as sg, \
                 tc.tile_pool(name="sWq", bufs=2) as swq, \
                 tc.tile_pool(name="sPr", bufs=2) as spr, \
                 tc.tile_pool(name="sP16", bufs=2) as sp16, \
                 tc.tile_pool(name="sWin", bufs=2) as swin, \
                 tc.tile_pool(name="sGru", bufs=1) as sgru, \
                 tc.tile_pool(name="sA2", bufs=2) as sa2, \
                 tc.tile_pool(name="sFt", bufs=2) as sft, \
                 tc.tile_pool(name="sSt", bufs=1) as sst:

                def distribute_h(hT, step):
                    """transpose hT [D, npc] -> [npc, D] rows, DMA to cc_in,
                    AllGather into h_fulls[step]."""
                    hst = sst.tile([128, wpc, D], F16, tag="hst")
                    for w in range(wpc):
                        ps2 = ppt.tile([128, 128], F32, tag="ps2")
                        nc.tensor.transpose(ps2[:, 0:D],
                                            hT[:, w * 128:(w + 1) * 128],
                                            idm[0:D, 0:D])
                        nc.scalar.copy(hst[:, w, :], ps2[:, 0:D])
                    nc.sync.dma_start(
                        cc_ins[step][:, 0:D].rearrange("(w p) d -> p w d", p=128),
                        hst[:])
                    nc.gpsimd.collective_compute(
                        "AllGather", OP.bypass,
                        replica_groups=[list(range(ncores))],
                        ins=[cc_ins[step][:].opt()],
                        outs=[h_fulls[step][:].opt()])

                def issue_gathers(G, step):
                    bnds = [round(i * ntpc / NSPLIT) for i in range(NSPLIT + 1)]
                    for i, (ta, tb) in enumerate(zip(bnds[:-1], bnds[1:])):
                        if tb > ta:
                            nc.gpsimd.dma_gather(
                                G[:, ta:tb, :], h_fulls[step][:, :],
                                isrc[:, ta * 8:tb * 8],
                                (tb - ta) * 128, (tb - ta) * 128, 128,
                                transpose=False, single_packet=False,
                                queue_num=i % NQ)

                def scatter_tile(gt, p16, j, aT, state):
                    """accumulate tile gt into its PSUM window; on window end
                    run the i-reduction tree + transpose into aT.  Returns the
                    completed window index (or None)."""
                    w = gt // T
                    tloc = gt % T
                    if tloc == 0:
                        psa_new = psa_pool.tile([128, 512], F32, tag="psa")
                        state["psa"] = psa_new
                    psa = state["psa"]
                    nc.tensor.matmul(
                        psa[:], lhsT=S_sb[:, gt * 128:(gt + 1) * 128],
                        rhs=p16[:, j, :, :],
                        start=(tloc == 0), stop=(tloc == T - 1))
                    if tloc != T - 1:
                        return None
                    aw = swin.tile([128, D, 16], F32, tag="aw")
                    nc.scalar.copy(
                        aw[:], psa[:].rearrange("p (o i) -> p o i", o=D))
                    t8 = swin.tile([128, D, 8], F32, tag="t8")
                    nc.vector.tensor_tensor(t8[:], aw[:, :, 0:8],
                                            aw[:, :, 8:16], op=OP.add)
                    t4 = swin.tile([128, D, 4], F32, tag="t4")
                    nc.vector.tensor_tensor(t4[:], t8[:, :, 0:4],
                                            t8[:, :, 4:8], op=OP.add)
                    t2 = swin.tile([128, D, 2], F32, tag="t2")
                    nc.vector.tensor_tensor(t2[:], t4[:, :, 0:2],
                                            t4[:, :, 2:4], op=OP.add)
                    t1 = swin.tile([128, D], F32, tag="t1")
                    nc.vector.tensor_tensor(t1[:], t2[:, :, 0],
                                            t2[:, :, 1], op=OP.add)
                    pst = ppt.tile([D, 128], F32, tag="pst")
                    nc.tensor.transpose(pst[:], t1[:], idm[:])
                    nc.vector.tensor_copy(
                        aT[:, w * 128:(w + 1) * 128], pst[:])
                    return w

                def gru_chunk(aT, h_cur, h_new, c0, cn):
                    if True:
                        cs = slice(c0, c0 + cn)
                        pgi = pmm.tile([3 * D, 512], F32, tag="mm96")
                        nc.tensor.matmul(pgi[:, 0:cn], lhsT=wih_sb[:],
                                         rhs=aT[:, cs], start=True, stop=True)
                        pgh = pmm.tile([3 * D, 512], F32, tag="mm96")
                        nc.tensor.matmul(pgh[:, 0:cn], lhsT=whh_sb[:],
                                         rhs=h_cur[:, cs], start=True, stop=True)
                        gh_sb = sgru.tile([3 * D, 512], F32, tag="gh_sb")
                        nc.scalar.copy(gh_sb[:, 0:cn], pgh[:, 0:cn])
                        tr = sgru.tile([D, 512], F32, tag="tr")
                        nc.vector.tensor_add(tr[:, 0:cn], pgi[0:D, 0:cn],
                                             gh_sb[0:D, 0:cn])
                        r = sgru.tile([D, 512], F32, tag="r")
                        nc.scalar.activation(r[:, 0:cn], tr[:, 0:cn], AF.Sigmoid,
                                             bias=br_sb[:])
                        tz = sgru.tile([D, 512], F32, tag="tz")
                        nc.vector.tensor_add(tz[:, 0:cn], pgi[D:2 * D, 0:cn],
                                             gh_sb[D:2 * D, 0:cn])
                        z = sgru.tile([D, 512], F32, tag="z")
                        nc.scalar.activation(z[:, 0:cn], tz[:, 0:cn], AF.Sigmoid,
                                             bias=bz_sb[:])
                        hnb = sgru.tile([D, 512], F32, tag="hnb")
                        nc.vector.tensor_scalar_add(hnb[:, 0:cn],
                                                    gh_sb[2 * D:3 * D, 0:cn],
                                                    bhn_sb[:])
                        rhn = sgru.tile([D, 512], F32, tag="rhn")
                        nc.vector.tensor_mul(rhn[:, 0:cn], r[:, 0:cn], hnb[:, 0:cn])
                        tn_ = sgru.tile([D, 512], F32, tag="tn_")
                        nc.vector.tensor_add(tn_[:, 0:cn], rhn[:, 0:cn],
                                             pgi[2 * D:3 * D, 0:cn])
                        ngate = sgru.tile([D, 512], F32, tag="ngate")
                        nc.scalar.activation(ngate[:, 0:cn], tn_[:, 0:cn], AF.Tanh,
                                             bias=bin_sb[:])
                        hmn = sgru.tile([D, 512], F32, tag="hmn")
                        nc.vector.tensor_sub(hmn[:, 0:cn], h_cur[:, cs],
                                             ngate[:, 0:cn])
                        zh = sgru.tile([D, 512], F32, tag="zh")
                        nc.vector.tensor_mul(zh[:, 0:cn], z[:, 0:cn], hmn[:, 0:cn])
                        nc.vector.tensor_add(h_new[:, cs], ngate[:, 0:cn],
                                             zh[:, 0:cn])

                def on_window_done(w, aT, h_cur, h_new):
                    """run the GRU for a 512-col slab once its 4 windows of
                    aT are final (keeps the GRU off the step tail)."""
                    if w is None:
                        return
                    if (w + 1) % 4 == 0:
                        gru_chunk(aT, h_cur, h_new, (w - 3) * 128, 512)
                    elif w == wpc - 1 and wpc % 4:
                        gru_chunk(aT, h_cur, h_new, (wpc - wpc % 4) * 128,
                                  (wpc % 4) * 128)

                def p16_add(p16, prod, k):
                    eng = nc.gpsimd if P16_POOL else nc.vector
                    eng.tensor_tensor(p16[:, 0:k, :, :], prod[:, 0:k, :, 0:16],
                                      prod[:, 0:k, :, 16:32], op=OP.add)

                # ---------------- Phase A1: own-shard h0T + AllGather -------
                for c0 in range(0, npc, 512):
                    cn = min(512, npc - c0)
                    ps = pmm.tile([3 * D, 512], F32, tag="mm96")
                    nc.tensor.matmul(ps[0:D, 0:cn], lhsT=wemb_sb[:],
                                     rhs=xTo_sb[:, c0:c0 + cn],
                                     start=True, stop=True)
                    nc.vector.tensor_copy(h_bufs[0][:, c0:c0 + cn],
                                          ps[0:D, 0:cn])
                distribute_h(h_bufs[0], 0)

                # ---------------- Step 0 (fused edge-MLP + messages) --------
                G = sg.tile([128, ntpc, 128], F16, tag="G")
                issue_gathers(G, 0)

                aT = sgru.tile([D, npc], F32, tag="aT")
                state = {}
                evac_flip = 0
                for q0 in range(0, ntpc, CT):
                    k = min(CT, ntpc - q0)
                    wt = swq.tile([128, CT, 1024], F16, tag="wq")
                    # edge MLP for this chunk (k*128 edges, groups of 512)
                    for g0 in range(0, k * 128, 512):
                        gn = min(512, k * 128 - g0)
                        e0 = q0 * 128 + g0
                        fts = sft.tile([5, 512], F16, tag="fts")
                        nc.sync.dma_start(fts[:, 0:gn], t_fT.ap()[:, e0:e0 + gn])
                        psh = pmm.tile([3 * D, 512], F32, tag="mm96")
                        nc.tensor.matmul(psh[0:D, 0:gn], lhsT=wf_sb[:],
                                         rhs=fts[:, 0:gn], start=True, stop=True)
                        he = sa2.tile([D, 512], F16, tag="he")
                        nc.scalar.activation(he[:, 0:gn], psh[0:D, 0:gn], AF.Relu)
                        psg = pmm.tile([3 * D, 512], F32, tag="mm96")
                        nc.tensor.matmul(psg[0:64, 0:gn], lhsT=w1_sb[:],
                                         rhs=he[:, 0:gn], start=True, stop=True)
                        ga = sa2.tile([65, 512], F16, tag="ga")
                        nc.vector.memset(ga[64:65, 0:gn], 1.0)
                        nc.scalar.activation(ga[0:64, 0:gn], psg[0:64, 0:gn],
                                             AF.Relu, bias=b1c_sb[:])
                        for s0 in range(0, gn, 128):
                            jt = (g0 + s0) // 128  # tile within chunk
                            for half in range(2):
                                pwt = pw.tile([128, 512], F32, tag="pw")
                                nc.tensor.matmul(
                                    pwt[:], lhsT=ga[:, s0:s0 + 128],
                                    rhs=w2_sb[:, half * 512:(half + 1) * 512],
                                    start=True, stop=True)
                                dst_ap = wt[:, jt, half * 512:(half + 1) * 512]
                                if evac_flip == 0:
                                    nc.scalar.copy(dst_ap, pwt[:])
                                else:
                                    nc.vector.tensor_copy(dst_ap, pwt[:])
                                evac_flip ^= 1
                    nc.sync.dma_start(we_view[:, q0:q0 + k, :], wt[:, 0:k, :])
                    # message path for this chunk
                    prod = spr.tile([128, CT, D, D], F16, tag="prod")
                    in1 = _bcast_mid(G[:, q0:q0 + k, 0:D], D)
                    nc.vector.tensor_tensor(
                        prod[:, 0:k, :, :],
                        wt[:, 0:k, :].rearrange("p t (o i) -> p t o i", o=D),
                        in1, op=OP.mult)
                    p16 = sp16.tile([128, CT, D, 16], F16, tag="p16")
                    p16_add(p16, prod, k)
                    for j in range(k):
                        wdone = scatter_tile(q0 + j, p16, j, aT, state)
                        on_window_done(wdone, aT, h_bufs[0], h_bufs[1])

                if steps_exec > 1:
                    distribute_h(h_bufs[1], 1)

                # ---------------- Steps 1..: stream We from DRAM ------------
                for step in range(1, steps_exec):
                    h_cur = h_bufs[step % 2]
                    h_new = h_bufs[(step + 1) % 2]

                    G = sg.tile([128, ntpc, 128], F16, tag="G")
                    issue_gathers(G, step)

                    aT = sgru.tile([D, npc], F32, tag="aT")
                    state = {}
                    for q0 in range(0, ntpc, CT):
                        k = min(CT, ntpc - q0)
                        wq = swq.tile([128, CT, 1024], F16, tag="wq")
                        nc.sync.dma_start(wq[:, 0:k, :], we_view[:, q0:q0 + k, :])
                        prod = spr.tile([128, CT, D, D], F16, tag="prod")
                        in1 = _bcast_mid(G[:, q0:q0 + k, 0:D], D)
                        nc.vector.tensor_tensor(
                            prod[:, 0:k, :, :],
                            wq[:, 0:k, :].rearrange("p t (o i) -> p t o i", o=D),
                            in1, op=OP.mult)
                        p16 = sp16.tile([128, CT, D, 16], F16, tag="p16")
                        p16_add(p16, prod, k)
                        for j in range(k):
                            wdone = scatter_tile(q0 + j, p16, j, aT, state)
                            on_window_done(wdone, aT, h_cur, h_new)

                    if step < steps_exec - 1:
                        distribute_h(h_new, step + 1)

                # ---------------- final store -------------------------------
                h_fin = h_bufs[steps_exec % 2]
                ost = sgru.tile([128, wpc, D], F32, tag="ost")
                for w in range(wpc):
                    ps2 = ppt.tile([128, 128], F32, tag="ps2")
                    nc.tensor.transpose(ps2[:, 0:D],
                                        h_fin[:, w * 128:(w + 1) * 128],
                                        idm[0:D, 0:D])
                    nc.scalar.copy(ost[:, w, :], ps2[:, 0:D])
                nc.sync.dma_start(
                    t_out.ap().rearrange("(w p) d -> p w d", p=128), ost[:])

    nc.compile()
    return nc


# --------------------------------------------------------------------------- #
# entry point
# --------------------------------------------------------------------------- #

def run(inputs, n_nodes=N_NODES, npc=NPC, **spmd_kwargs):
    global LAST_RESULT, LAST_META
    shared, per_core, meta, perm = host_prep(**inputs, n_nodes=n_nodes, npc=npc)
    LAST_META = meta
    nc = build_nc(meta)
    in_maps = [dict(shared, **pc) for pc in per_core]
    res = bass_utils.run_bass_kernel_spmd(
        nc, in_maps, core_ids=list(range(meta["ncores"])), **spmd_kwargs)
    LAST_RESULT = res
    out = np.concatenate([res.results[c]["out_h"] for c in range(meta["ncores"])], 0)
    return np.ascontiguousarray(out[perm]).astype(np.float32)


def kernel(**inputs):
    return run(inputs)
